# revision 47
# baseline (speedup 1.0000x reference)
"""Masked multi-head self-attention on 8 Trainium2 NeuronCores.

Sharding: core c handles batch b = c // 2 and head-group g = c % 2
(8 of 16 heads).  Data-parallel over B, tensor-parallel over heads for
qkv_proj (column split) / out_proj (row split).  The [T,T] causal mask
is exploited structurally (tile skipping); the host verifies the mask
is causal and falls back to numpy otherwise.  Host sums the two
head-group partial outputs per batch and adds bout.

Schedule: projection matmul groups are interleaved into the attention
stream so TensorE works through softmax (ScalarE) stretches.  The
softmax denominators ride free on the PV matmuls: each 64-wide V
stationary is widened to 128 with 64 all-ones columns (matmul cost
depends only on moving free size), so rows 64:128 of the PV psum hold
the denominator replicated across partitions.  The per-pair tail is
then a DVE-only chain - partition-parallel reciprocal + multiply into
the per-(pair,chunk) attention tiles - with no extra TensorE work and
no cross-partition broadcast.
"""

import numpy as np
import ml_dtypes

BF16 = ml_dtypes.bfloat16

B = 4
T = 2048
D = 1024
H = 16
DK = 64
P = 128
NCORES = 8

KT = D // P            # 8   k-tiles over d_model
TTILES = T // P        # 16  tiles over tokens
NCH = 4                # qi chunks of 512
CH = T // NCH          # 512

_CACHE = {}


def _build_program(with_bias=True):
    import concourse.bass as bass
    import concourse.tile as tile
    from concourse import bacc, mybir
    from contextlib import ExitStack

    f32 = mybir.dt.float32
    bf16 = mybir.dt.bfloat16
    nc = bacc.Bacc("TRN2", target_bir_lowering=False, debug=False,
                   num_devices=NCORES)

    xt_d = nc.declare_dram_parameter("xt", [P, KT * T], bf16, isOutput=False)
    wqk_d = nc.declare_dram_parameter("wqk", [P, 8 * 1024], bf16, isOutput=False)
    wv_d = nc.declare_dram_parameter("wv", [P, KT * 512], bf16, isOutput=False)
    wout_d = nc.declare_dram_parameter("wout", [P, 4 * 1024], bf16, isOutput=False)
    m01_d = nc.declare_dram_parameter("m01", [P, P], bf16, isOutput=False)
    bqk_d = nc.declare_dram_parameter("bqk", [P, 8], f32, isOutput=False)
    bv_d = nc.declare_dram_parameter("bv", [1, 512], bf16, isOutput=False)
    out_d = nc.declare_dram_parameter("out", [T, D], bf16, isOutput=True)

    ts = bass.ts
    EXP = mybir.ActivationFunctionType.Exp

    with tile.TileContext(nc) as tc, ExitStack() as top:
        const = top.enter_context(tc.tile_pool(name="const", bufs=1))
        big = top.enter_context(tc.tile_pool(name="big", bufs=1))
        wqk_pool = top.enter_context(tc.tile_pool(name="wqk", bufs=8))
        pt_pool = top.enter_context(tc.tile_pool(name="pt", bufs=10))
        rs_pool = top.enter_context(tc.tile_pool(name="rs", bufs=2))
        osb_pool = top.enter_context(tc.tile_pool(name="osb", bufs=4))
        oprt_pool = top.enter_context(tc.tile_pool(name="oprt", bufs=8))
        # PSUM: "s" 2x[128,1024]f32 = 4 banks, "avr" 2x[128,512] = 2, "q" 2
        ps_s = top.enter_context(tc.tile_pool(name="ps_s", bufs=2, space="PSUM"))
        ps_avr = top.enter_context(tc.tile_pool(name="ps_avr", bufs=2, space="PSUM"))
        ps_q = top.enter_context(tc.tile_pool(name="ps_q", bufs=2, space="PSUM"))

        ones_row = const.tile([1, P], bf16, tag="ones_row")
        neg12 = const.tile([P, 1], f32, tag="neg12")
        bqk_sb = const.tile([P, 8], f32, tag="bqk")
        bv_sb = const.tile([1, 512], bf16, tag="bv")
        m01_blk = const.tile([P, P], bf16, tag="m01")
        nc.vector.memset(ones_row[:], 1.0)
        nc.vector.memset(neg12[:], -12.0)

        def qk_evac(dst, acc, i):
            if with_bias:
                nc.vector.tensor_scalar_add(dst, acc, bqk_sb[:, i:i + 1])
            else:
                nc.vector.tensor_copy(dst, acc)

        # qk[i] for i<4: q of head pair i (pre-scaled 1/8); i>=4: k of pair i-4
        # v_all: per token tile, 8 head slots of 128 cols = [v_h | ones x64];
        # stationary width is free, so each PV matmul also produces the
        # softmax denominator replicated across partitions 64:128
        VW = 8 * 2 * DK  # 1024
        xt = big.tile([P, KT * T], bf16, tag="xt")
        qk = [big.tile([P, T], bf16, tag=f"qk{i}", name=f"qk{i}")
              for i in range(8)]
        v_all = big.tile([P, TTILES * VW], bf16, tag="v")
        # at[p][c]: one tile per (head-pair, token-chunk) so out-proj reads
        # depend only on the pair tail that actually wrote them
        at = [[big.tile([P, CH], bf16, tag=f"at{p}_{c}", name=f"at{p}_{c}")
               for c in range(NCH)] for p in range(4)]
        wv_sb = big.tile([P, KT * 512], bf16, tag="wv")
        wout_sb = big.tile([P, 4 * 1024], bf16, tag="wout")
        v4 = v_all[:].rearrange("p (t h c) -> p (t h) c", h=8, c=2 * DK)

        wqk_sb = {}

        def dma_wqk(i, eng=None, split=False):
            w = wqk_pool.tile([P, 1024], bf16, tag="wqk", name=f"wqk{i}")
            e = eng or nc.sync
            if split:
                e.dma_start(w[:, 0:512], wqk_d[:, i * 1024: i * 1024 + 512])
                e.dma_start(w[:, 512:1024],
                            wqk_d[:, i * 1024 + 512: (i + 1) * 1024])
            else:
                e.dma_start(w[:], wqk_d[:, ts(i, 1024)])
            wqk_sb[i] = w

        if with_bias:
            nc.sync.dma_start(bqk_sb[:], bqk_d[:])
        # DMA program follows matmul consumption order, interleaved on both
        # HWDGE queues: qk(0,0)h0 wants wqk0+quarters 0-3, qk(4,0) wants
        # wqk4, the v-prelude wants wv, exp(0,0,0) wants m01, then chunk-0
        # fills want wqk1/5, chunk-1+ wants the xt second halves + wqk2/6/3/7
        dma_wqk(0)
        for kt in range(4):
            eng = nc.scalar if kt % 2 == 0 else nc.sync
            eng.dma_start(xt[:, kt * T: kt * T + 512],
                          xt_d[:, kt * T: kt * T + 512])
        dma_wqk(4, nc.scalar)
        for kt in range(4, KT):
            eng = nc.scalar if kt % 2 == 0 else nc.sync
            eng.dma_start(xt[:, kt * T: kt * T + 512],
                          xt_d[:, kt * T: kt * T + 512])
        nc.sync.dma_start(wv_sb[:, 0:4 * 512], wv_d[:, 0:4 * 512])
        nc.scalar.dma_start(wv_sb[:, 4 * 512:], wv_d[:, 4 * 512:])
        nc.sync.dma_start(m01_blk[:], m01_d[:])
        if with_bias:
            nc.sync.dma_start(bv_sb[:], bv_d[:])
        dma_wqk(1)
        dma_wqk(5, nc.scalar)
        for kt in range(KT):
            eng = nc.scalar if kt % 2 == 0 else nc.sync
            eng.dma_start(xt[:, kt * T + 512: (kt + 1) * T],
                          xt_d[:, kt * T + 512: (kt + 1) * T])
        nc.sync.dma_start(wout_sb[:], wout_d[:])
        dma_wqk(2)
        dma_wqk(6, nc.scalar)
        dma_wqk(3)
        dma_wqk(7, nc.scalar)

        def emit_qk_chunk(i, n, half=None, cell=None):
            """half=None: whole 8-matmul chain; half=0/1: split granule
            (the two granules share one psum tile via `cell`)."""
            if half in (None, 0):
                acc = ps_q.tile([P, 512], f32, tag="q", name="qkacc")
                if cell is not None:
                    cell.append(acc)
            else:
                acc = cell.pop()
            k0 = 0 if half in (None, 0) else KT // 2
            k1 = KT if half in (None, 1) else KT // 2
            for kt in range(k0, k1):
                nc.tensor.matmul(
                    acc[:], wqk_sb[i][:, ts(kt, P)],
                    xt[:, kt * T + n * CH: kt * T + (n + 1) * CH],
                    start=(kt == 0), stop=(kt == KT - 1))
            if half in (None, 1):
                qk_evac(qk[i][:, ts(n, CH)], acc[:], i)

        def emit_v_merged(t2):
            """v token-tiles 2*t2, 2*t2+1 in one 2-bank psum tile"""
            acc = ps_s.tile([P, 1024], f32, tag="s", name="vacc")
            for kt in range(KT):
                for h in range(2):
                    t = 2 * t2 + h
                    nc.tensor.matmul(
                        acc[:, ts(h, 512)],
                        xt[:, kt * T + t * P: kt * T + (t + 1) * P],
                        wv_sb[:, ts(kt, 512)],
                        start=(kt == 0),
                        stop=(not with_bias and kt == KT - 1))
            if with_bias:
                for h in range(2):
                    nc.tensor.matmul(acc[:, ts(h, 512)], ones_row[:],
                                     bv_sb[:], start=False, stop=True)
            nc.vector.tensor_copy(
                v_all[:, 2 * t2 * VW:(2 * t2 + 2) * VW].rearrange(
                    "p (t h c) -> p t h c", h=8, c=2 * DK)[:, :, :, 0:DK],
                acc[:].rearrange("p (t h d) -> p t h d", h=8, d=DK))

        def emit_v_chunk(t, half=None, cell=None):
            if half in (None, 0):
                acc = ps_q.tile([P, 512], f32, tag="q", name="vacc")
                if cell is not None:
                    cell.append(acc)
            else:
                acc = cell.pop()
            k0 = 0 if half in (None, 0) else KT // 2
            k1 = KT if half in (None, 1) else KT // 2
            for kt in range(k0, k1):
                nc.tensor.matmul(
                    acc[:], xt[:, kt * T + t * P: kt * T + (t + 1) * P],
                    wv_sb[:, ts(kt, 512)],
                    start=(kt == 0),
                    stop=(not with_bias and kt == KT - 1))
            if half in (None, 1):
                if with_bias:
                    nc.tensor.matmul(acc[:], ones_row[:], bv_sb[:],
                                     start=False, stop=True)
                nc.vector.tensor_copy(
                    v_all[:, t * VW:(t + 1) * VW].rearrange(
                        "p (h c) -> p h c", h=8, c=2 * DK)[:, :, 0:DK],
                    acc[:].rearrange("p (h d) -> p h d", h=8, d=DK))

        op_parts = {}

        def emit_op(t, dc, half=None, cell=None, sc_evac=False):
            """out-projection for token tile t, output column half dc.
            half=0: kk 0..2 accumulated and parked in sbuf (bf16 partial);
            half=1: kk 3 matmul + add partial + evac.  Lets the bulk of the
            final tiles' work run before the last pair's normalize lands."""
            acc = ps_q.tile([P, 512], f32, tag="q", name="oacc")
            kks = range(4) if half is None else (
                range(3) if half == 0 else range(3, 4))
            for kk in kks:
                nc.tensor.matmul(
                    acc[:],
                    at[kk][t // 4][:, (t % 4) * P:(t % 4 + 1) * P],
                    wout_sb[:, kk * 1024 + dc * 512: kk * 1024 + dc * 512 + 512],
                    start=(kk == 0 or half == 1), stop=(kk == 3 or half == 0))
            if half == 0:
                part = oprt_pool.tile([P, 512], bf16, tag="op_part",
                                      name=f"part{t}_{dc}")
                nc.scalar.activation(part[:], acc[:],
                                     mybir.ActivationFunctionType.Identity)
                op_parts[(t, dc)] = part
                return
            o_sb = osb_pool.tile([P, 512], bf16, tag="o_sb")
            if half == 1:
                nc.vector.tensor_add(o_sb[:], acc[:], op_parts[(t, dc)][:])
            elif sc_evac:
                nc.scalar.activation(o_sb[:], acc[:],
                                     mybir.ActivationFunctionType.Identity)
            else:
                nc.vector.tensor_copy(o_sb[:], acc[:])
            nc.sync.dma_start(out_d[ts(t, P), ts(dc, 512)], o_sb[:])

        # ---- attention step machine ---------------------------------------
        # Globally software-pipelined: the scores for step i+1 (even across a
        # pair boundary) are issued while ScalarE computes exp(i), and fill
        # groups land inside the exp window, so neither engine waits.
        pair_ps = {}
        s_tiles = {}
        rec_tiles = {}

        def pair_begin(c, p):
            ta = ps_avr.tile([P, CH], f32, tag="avr", name="attnA")
            tb = ps_avr.tile([P, CH], f32, tag="avr", name="attnB")
            pair_ps[(c, p)] = (ta, tb)

        def emit_score(c, p, j):
            kq = qk[4 + p]
            qq = qk[p]
            off = max(0, P * (j - 4 * c))
            s_ps = ps_s.tile([P, 1024], f32, tag="s", name="s_ps")
            nc.tensor.matmul(
                s_ps[:, off:512], kq[0:DK, ts(j, P)],
                qq[0:DK, c * CH + off:(c + 1) * CH],
                start=True, stop=True)
            nc.tensor.matmul(
                s_ps[:, 512 + off:1024], kq[DK:P, ts(j, P)],
                qq[DK:P, c * CH + off:(c + 1) * CH],
                start=True, stop=True)
            s_tiles[(c, p, j)] = s_ps

        def emit_exp(c, p, j):
            off = max(0, P * (j - 4 * c))
            s_ps = s_tiles.pop((c, p, j))
            pt = pt_pool.tile([P, 1024], bf16, tag="pt")
            # [128, 2, n] views pairing the two head-halves (stride 512),
            # so trimmed exp / diagonal masking are single instructions
            pt3 = pt[:].rearrange("p (two n) -> p two n", two=2)
            s3 = s_ps[:].rearrange("p (two n) -> p two n", two=2)
            m3 = m01_blk[:, None, :].broadcast_to([P, 2, P])
            if j > 4 * c:  # diagonal block, trimmed
                nc.scalar.activation(
                    pt3[:, :, off:512], s3[:, :, off:512],
                    EXP, bias=neg12[:], scale=1.0)
                nc.vector.tensor_mul(
                    pt3[:, :, off:off + P], pt3[:, :, off:off + P], m3)
            elif j == 4 * c:  # diagonal block at chunk start
                nc.scalar.activation(
                    pt[:], s_ps[:], EXP, bias=neg12[:], scale=1.0)
                nc.vector.tensor_mul(
                    pt3[:, :, 0:P], pt3[:, :, 0:P], m3)
            else:
                nc.scalar.activation(
                    pt[:], s_ps[:], EXP, bias=neg12[:], scale=1.0)
            return pt

        def emit_pv(c, p, j, pt):
            ta, tb = pair_ps[(c, p)]
            nki = 4 * (c + 1)
            st = (j == 0)
            sp = (j == nki - 1)
            off = max(0, P * (j - 4 * c))
            vb = j * VW
            # [v_h | ones x64] stationary: rows 0:64 attn, rows 64:128 the
            # softmax denominator replicated across partitions
            nc.tensor.matmul(
                ta[:, off:CH],
                v_all[:, vb + 2 * p * P: vb + (2 * p + 1) * P],
                pt[:, off:512],
                start=st, stop=sp, skip_group_check=True)
            nc.tensor.matmul(
                tb[:, off:CH],
                v_all[:, vb + (2 * p + 1) * P: vb + (2 * p + 2) * P],
                pt[:, 512 + off:1024],
                start=st, stop=sp, skip_group_check=True)

        def emit_tail(c, p):
            """normalize: partition-parallel reciprocals of the replicated
            denominators, then multiply into at_all.  DVE-only chain, so it
            runs concurrently with the next pair's scores/fills.  The
            reciprocals must start at partition base 0 (custom-DVE ops
            silently ignore nonzero bases): rows 0:64 compute junk recips of
            attn values that nothing reads."""
            ta, tb = pair_ps.pop((c, p))
            rs_sb = rs_pool.tile([P, 2 * CH], f32, tag="rs_sb")
            dst = at[p][c]
            nc.vector.reciprocal_approx_fast(rs_sb[:, 0:CH], ta[:, :])
            nc.vector.tensor_mul(dst[0:DK, :], ta[0:DK, :],
                                 rs_sb[DK:P, 0:CH])
            nc.vector.reciprocal_approx_fast(rs_sb[:, CH:2 * CH], tb[:, :])
            nc.vector.tensor_mul(dst[DK:P, :], tb[0:DK, :],
                                 rs_sb[DK:P, CH:2 * CH])

        def qkf(i, n):
            """two ~1us granules sharing one psum accumulator"""
            cell = []
            return [lambda h=h: emit_qk_chunk(i, n, half=h, cell=cell)
                    for h in range(2)]

        def vf(t):
            cell = []
            return [lambda h=h: emit_v_chunk(t, half=h, cell=cell)
                    for h in range(2)]

        def opf(tt):
            return [lambda t=t, dc=dc: emit_op(t, dc)
                    for t in tt for dc in range(2)]

        def opf_early(tt):
            return [lambda t=t, dc=dc: emit_op(t, dc, half=0)
                    for t in tt for dc in range(2)]

        # ---- prelude: only what pair (0,0) needs — chunk-0 of q0/k0 and
        # v tiles 0..3 (v accumulated in the idle "s" score pool) -----------
        nc.vector.memset(v4[:, :, DK:2 * DK], 1.0)
        cell0, cell4 = [], []
        emit_qk_chunk(0, 0, half=0, cell=cell0)
        emit_qk_chunk(4, 0, half=0, cell=cell4)
        emit_qk_chunk(0, 0, half=1, cell=cell0)
        emit_qk_chunk(4, 0, half=1, cell=cell4)

        # ---- main interleave.  qk projection chunk n of tile i is first
        # needed by pair (n, i%4), so each pair carries its successor's two
        # qk chunk-groups; v/out-proj groups fill the remaining slack, with
        # out-proj (no early deadline) pushed into the late exp-bound
        # chunks. ----------------------------------------------------------
        fills = {
            (0, 0): vf(2) + vf(3) + qkf(1, 0) + qkf(5, 0),
            (0, 1): qkf(2, 0) + qkf(6, 0),
            (0, 2): qkf(3, 0) + qkf(7, 0),
            (0, 3): qkf(0, 1) + qkf(4, 1) + vf(4) + vf(5) + vf(6) + vf(7),
            (1, 0): qkf(1, 1) + qkf(5, 1) + vf(8),
            (1, 1): qkf(2, 1) + qkf(6, 1) + vf(9),
            (1, 2): qkf(3, 1) + qkf(7, 1) + vf(10),
            (1, 3): qkf(0, 2) + qkf(4, 2) + vf(11),
            (2, 0): qkf(1, 2) + qkf(5, 2) + opf([0]),
            (2, 1): qkf(2, 2) + qkf(6, 2) + opf([1]),
            (2, 2): qkf(3, 2) + qkf(7, 2) + opf([2, 3]),
            (2, 3): qkf(0, 3) + qkf(4, 3) + vf(12) + vf(13) + opf([4]),
            (3, 0): vf(14) + vf(15) + qkf(1, 3) + qkf(5, 3) + opf([5]),
            (3, 1): qkf(2, 3) + qkf(6, 3) + opf([6]),
            (3, 2): qkf(3, 3) + qkf(7, 3) + opf([7, 8]),
            (3, 3): opf([9, 10, 11]),
        }
        steps = [(c, p, j)
                 for c in range(NCH) for p in range(4)
                 for j in range(4 * (c + 1))]
        pair_begin(0, 0)
        emit_score(0, 0, 0)
        emit_v_merged(0)
        fcur = {}
        for idx, (c, p, j) in enumerate(steps):
            nki = 4 * (c + 1)
            pt = emit_exp(c, p, j)
            if idx + 1 < len(steps):
                nc2, np2, nj2 = steps[idx + 1]
                if nj2 == 0:
                    pair_begin(nc2, np2)
                emit_score(nc2, np2, nj2)
            # fills must complete by step nki-2: the last step's lookahead
            # score reads qk chunks that this pair's fills produce
            fl = fills.get((c, p), ())
            want = min(len(fl), max((j + 2) * len(fl) // nki,
                                    1 if j == 0 else 0))
            cur = fcur.get((c, p), 0)
            while cur < want:
                fl[cur]()
                cur += 1
            fcur[(c, p)] = cur
            emit_pv(c, p, j, pt)
            if j == nki - 1:
                emit_tail(c, p)
        for t in range(12, 16):
            for dc in range(2):
                emit_op(t, dc, sc_evac=True)

    nc.compile()
    return nc


def _get_program(with_bias):
    key = ("nc", with_bias)
    if key not in _CACHE:
        _CACHE[key] = _build_program(with_bias)
    return _CACHE[key]


def _prep_core_inputs(x, attn_mask, Wqkv, bqkv, Wout):
    """Per-core host-side sharding + DMA-friendly layouts."""
    # partial diagonal block: m01[ki_rel, qi_rel] = 1 iff qi_rel >= ki_rel
    m01 = np.triu(np.ones((P, P), np.float32)).astype(BF16)

    in_maps = []
    for core in range(NCORES):
        b, g = core // 2, core % 2
        xt = np.ascontiguousarray(
            x[b].T.reshape(KT, P, T).transpose(1, 0, 2).reshape(P, KT * T)
        ).astype(BF16)
        wq = Wqkv[:, 512 * g:512 * g + 512] * np.float32(0.125)
        wk = Wqkv[:, 1024 + 512 * g:1024 + 512 * g + 512]
        wqk = np.concatenate([wq, wk], axis=1)  # [1024, 1024]
        wqk = np.ascontiguousarray(
            wqk.reshape(KT, P, 8, P).transpose(1, 2, 0, 3).reshape(P, 8192)
        ).astype(BF16)
        wv = Wqkv[:, 2048 + 512 * g:2048 + 512 * g + 512]
        wv = np.ascontiguousarray(
            wv.reshape(KT, P, 512).transpose(1, 0, 2).reshape(P, KT * 512)
        ).astype(BF16)
        wo = Wout[512 * g:512 * g + 512, :]
        wo = np.ascontiguousarray(
            wo.reshape(4, P, 1024).transpose(1, 0, 2).reshape(P, 4096)
        ).astype(BF16)
        bq = bqkv[512 * g:512 * g + 512] * np.float32(0.125)
        bk = bqkv[1024 + 512 * g:1024 + 512 * g + 512]
        bqk = np.ascontiguousarray(
            np.concatenate([bq, bk]).reshape(8, P).T)
        bv = np.ascontiguousarray(
            bqkv[2048 + 512 * g:2048 + 512 * g + 512].reshape(1, 512)
        ).astype(BF16)
        in_maps.append({"xt": xt, "wqk": wqk, "wv": wv, "wout": wo,
                        "m01": m01, "bqk": bqk, "bv": bv})
    return in_maps


def _mask_is_causal(attn_mask):
    zero = (attn_mask == 0.0)
    if not np.array_equal(zero, np.tril(np.ones((T, T), dtype=bool))):
        return False
    return bool(np.all(attn_mask[~zero] <= np.float32(-50.0)))


def _numpy_fallback(x, attn_mask, Wqkv, bqkv, Wout, bout):
    qkv = x @ Wqkv + bqkv
    qkv = qkv.reshape(B, T, 3, H, DK).transpose(2, 0, 3, 1, 4)
    q, k, vv = qkv[0], qkv[1], qkv[2]
    scores = np.einsum("bhqd,bhkd->bhqk", q, k) / np.float32(np.sqrt(DK))
    scores = scores + attn_mask
    scores -= scores.max(axis=-1, keepdims=True)
    e = np.exp(scores)
    probs = e / e.sum(axis=-1, keepdims=True)
    attn = np.einsum("bhqk,bhkd->bhqd", probs, vv)
    attn = attn.transpose(0, 2, 1, 3).reshape(B, T, D)
    return (attn @ Wout + bout).astype(np.float32)


def _run(inputs, trace=False):
    from concourse.bass_utils import run_bass_kernel_spmd

    x = np.asarray(inputs["x"], dtype=np.float32)
    attn_mask = np.asarray(inputs["attn_mask"], dtype=np.float32)
    Wqkv = np.asarray(inputs["Wqkv"], dtype=np.float32)
    bqkv = np.asarray(inputs["bqkv"], dtype=np.float32)
    Wout = np.asarray(inputs["Wout"], dtype=np.float32)
    bout = np.asarray(inputs["bout"], dtype=np.float32)

    if not _mask_is_causal(attn_mask):
        return _numpy_fallback(x, attn_mask, Wqkv, bqkv, Wout, bout), None

    with_bias = bool(np.any(bqkv != 0.0))
    nc = _get_program(with_bias)
    in_maps = _prep_core_inputs(x, attn_mask, Wqkv, bqkv, Wout)
    res = run_bass_kernel_spmd(nc, in_maps, list(range(NCORES)), trace=trace)
    out = np.empty((B, T, D), np.float32)
    for b in range(B):
        out[b] = (res.results[2 * b]["out"].astype(np.float32)
                  + res.results[2 * b + 1]["out"].astype(np.float32) + bout)
    return out, res.exec_time_ns


def kernel(**inputs) -> np.ndarray:
    out, _ = _run(inputs, trace=False)
    return out



# revision 48
# speedup vs baseline: 1.0018x; 1.0018x over previous
"""Masked multi-head self-attention on 8 Trainium2 NeuronCores.

Sharding: core c handles batch b = c // 2 and head-group g = c % 2
(8 of 16 heads).  Data-parallel over B, tensor-parallel over heads for
qkv_proj (column split) / out_proj (row split).  The [T,T] causal mask
is exploited structurally (tile skipping); the host verifies the mask
is causal and falls back to numpy otherwise.  Host sums the two
head-group partial outputs per batch and adds bout.

Schedule: projection matmul groups are interleaved into the attention
stream so TensorE works through softmax (ScalarE) stretches.  The
softmax denominators ride free on the PV matmuls: each 64-wide V
stationary is widened to 128 with 64 all-ones columns (matmul cost
depends only on moving free size), so rows 64:128 of the PV psum hold
the denominator replicated across partitions.  The per-pair tail is
then a DVE-only chain - partition-parallel reciprocal + multiply into
the per-(pair,chunk) attention tiles - with no extra TensorE work and
no cross-partition broadcast.
"""

import numpy as np
import ml_dtypes

BF16 = ml_dtypes.bfloat16

B = 4
T = 2048
D = 1024
H = 16
DK = 64
P = 128
NCORES = 8

KT = D // P            # 8   k-tiles over d_model
TTILES = T // P        # 16  tiles over tokens
NCH = 4                # qi chunks of 512
CH = T // NCH          # 512

_CACHE = {}


def _build_program(with_bias=True):
    import concourse.bass as bass
    import concourse.tile as tile
    from concourse import bacc, mybir
    from contextlib import ExitStack

    f32 = mybir.dt.float32
    bf16 = mybir.dt.bfloat16
    nc = bacc.Bacc("TRN2", target_bir_lowering=False, debug=False,
                   num_devices=NCORES)

    xt_d = nc.declare_dram_parameter("xt", [P, KT * T], bf16, isOutput=False)
    wqk_d = nc.declare_dram_parameter("wqk", [P, 8 * 1024], bf16, isOutput=False)
    wv_d = nc.declare_dram_parameter("wv", [P, KT * 512], bf16, isOutput=False)
    wout_d = nc.declare_dram_parameter("wout", [P, 4 * 1024], bf16, isOutput=False)
    m01_d = nc.declare_dram_parameter("m01", [P, P], bf16, isOutput=False)
    bqk_d = nc.declare_dram_parameter("bqk", [P, 8], f32, isOutput=False)
    bv_d = nc.declare_dram_parameter("bv", [1, 512], bf16, isOutput=False)
    out_d = nc.declare_dram_parameter("out", [T, D], bf16, isOutput=True)

    ts = bass.ts
    EXP = mybir.ActivationFunctionType.Exp

    with tile.TileContext(nc) as tc, ExitStack() as top:
        const = top.enter_context(tc.tile_pool(name="const", bufs=1))
        big = top.enter_context(tc.tile_pool(name="big", bufs=1))
        wqk_pool = top.enter_context(tc.tile_pool(name="wqk", bufs=8))
        pt_pool = top.enter_context(tc.tile_pool(name="pt", bufs=10))
        rs_pool = top.enter_context(tc.tile_pool(name="rs", bufs=2))
        osb_pool = top.enter_context(tc.tile_pool(name="osb", bufs=4))
        oprt_pool = top.enter_context(tc.tile_pool(name="oprt", bufs=8))
        # PSUM: "s" 2x[128,1024]f32 = 4 banks, "avr" 2x[128,512] = 2, "q" 2
        ps_s = top.enter_context(tc.tile_pool(name="ps_s", bufs=2, space="PSUM"))
        ps_avr = top.enter_context(tc.tile_pool(name="ps_avr", bufs=2, space="PSUM"))
        ps_q = top.enter_context(tc.tile_pool(name="ps_q", bufs=2, space="PSUM"))

        ones_row = const.tile([1, P], bf16, tag="ones_row")
        neg12 = const.tile([P, 1], f32, tag="neg12")
        bqk_sb = const.tile([P, 8], f32, tag="bqk")
        bv_sb = const.tile([1, 512], bf16, tag="bv")
        m01_blk = const.tile([P, P], bf16, tag="m01")
        nc.vector.memset(ones_row[:], 1.0)
        nc.vector.memset(neg12[:], -12.0)

        def qk_evac(dst, acc, i):
            if with_bias:
                nc.vector.tensor_scalar_add(dst, acc, bqk_sb[:, i:i + 1])
            else:
                nc.vector.tensor_copy(dst, acc)

        # qk[i] for i<4: q of head pair i (pre-scaled 1/8); i>=4: k of pair i-4
        # v_all: per token tile, 8 head slots of 128 cols = [v_h | ones x64];
        # stationary width is free, so each PV matmul also produces the
        # softmax denominator replicated across partitions 64:128
        VW = 8 * 2 * DK  # 1024
        xt = big.tile([P, KT * T], bf16, tag="xt")
        qk = [big.tile([P, T], bf16, tag=f"qk{i}", name=f"qk{i}")
              for i in range(8)]
        v_all = big.tile([P, TTILES * VW], bf16, tag="v")
        # at[p][c]: one tile per (head-pair, token-chunk) so out-proj reads
        # depend only on the pair tail that actually wrote them
        at = [[big.tile([P, CH], bf16, tag=f"at{p}_{c}", name=f"at{p}_{c}")
               for c in range(NCH)] for p in range(4)]
        wv_sb = big.tile([P, KT * 512], bf16, tag="wv")
        wout_sb = big.tile([P, 4 * 1024], bf16, tag="wout")
        v4 = v_all[:].rearrange("p (t h c) -> p (t h) c", h=8, c=2 * DK)

        wqk_sb = {}

        def dma_wqk(i, eng=None, split=False):
            w = wqk_pool.tile([P, 1024], bf16, tag="wqk", name=f"wqk{i}")
            e = eng or nc.sync
            if split:
                e.dma_start(w[:, 0:512], wqk_d[:, i * 1024: i * 1024 + 512])
                e.dma_start(w[:, 512:1024],
                            wqk_d[:, i * 1024 + 512: (i + 1) * 1024])
            else:
                e.dma_start(w[:], wqk_d[:, ts(i, 1024)])
            wqk_sb[i] = w

        if with_bias:
            nc.sync.dma_start(bqk_sb[:], bqk_d[:])
        # DMA program follows matmul consumption order, interleaved on both
        # HWDGE queues: qk(0,0)h0 wants wqk0+quarters 0-3, qk(4,0) wants
        # wqk4, the v-prelude wants wv, exp(0,0,0) wants m01, then chunk-0
        # fills want wqk1/5, chunk-1+ wants the xt second halves + wqk2/6/3/7
        dma_wqk(0)
        for kt in range(4):
            eng = nc.scalar if kt % 2 == 0 else nc.sync
            eng.dma_start(xt[:, kt * T: kt * T + 512],
                          xt_d[:, kt * T: kt * T + 512])
        dma_wqk(4, nc.scalar)
        for kt in range(4, KT):
            eng = nc.scalar if kt % 2 == 0 else nc.sync
            eng.dma_start(xt[:, kt * T: kt * T + 512],
                          xt_d[:, kt * T: kt * T + 512])
        nc.sync.dma_start(wv_sb[:, 0:4 * 512], wv_d[:, 0:4 * 512])
        nc.scalar.dma_start(wv_sb[:, 4 * 512:], wv_d[:, 4 * 512:])
        nc.sync.dma_start(m01_blk[:], m01_d[:])
        if with_bias:
            nc.sync.dma_start(bv_sb[:], bv_d[:])
        dma_wqk(1)
        dma_wqk(5, nc.scalar)
        # xt second halves as two strided DMAs (even/odd k-blocks): frees
        # ~6 HWDGE issue slots so the later weight tiles trigger sooner
        xtv = xt[:].rearrange("p (k t) -> p k t", k=KT)
        xdv = xt_d[:].rearrange("p (k t) -> p k t", k=KT)
        nc.scalar.dma_start(xtv[:, 0::2, 512:T], xdv[:, 0::2, 512:T])
        nc.sync.dma_start(xtv[:, 1::2, 512:T], xdv[:, 1::2, 512:T])
        nc.sync.dma_start(wout_sb[:], wout_d[:])
        dma_wqk(2)
        dma_wqk(6, nc.scalar)
        dma_wqk(3)
        dma_wqk(7, nc.scalar)

        def emit_qk_chunk(i, n, half=None, cell=None):
            """half=None: whole 8-matmul chain; half=0/1: split granule
            (the two granules share one psum tile via `cell`)."""
            if half in (None, 0):
                acc = ps_q.tile([P, 512], f32, tag="q", name="qkacc")
                if cell is not None:
                    cell.append(acc)
            else:
                acc = cell.pop()
            k0 = 0 if half in (None, 0) else KT // 2
            k1 = KT if half in (None, 1) else KT // 2
            for kt in range(k0, k1):
                nc.tensor.matmul(
                    acc[:], wqk_sb[i][:, ts(kt, P)],
                    xt[:, kt * T + n * CH: kt * T + (n + 1) * CH],
                    start=(kt == 0), stop=(kt == KT - 1))
            if half in (None, 1):
                qk_evac(qk[i][:, ts(n, CH)], acc[:], i)

        def emit_v_merged(t2):
            """v token-tiles 2*t2, 2*t2+1 in one 2-bank psum tile"""
            acc = ps_s.tile([P, 1024], f32, tag="s", name="vacc")
            for kt in range(KT):
                for h in range(2):
                    t = 2 * t2 + h
                    nc.tensor.matmul(
                        acc[:, ts(h, 512)],
                        xt[:, kt * T + t * P: kt * T + (t + 1) * P],
                        wv_sb[:, ts(kt, 512)],
                        start=(kt == 0),
                        stop=(not with_bias and kt == KT - 1))
            if with_bias:
                for h in range(2):
                    nc.tensor.matmul(acc[:, ts(h, 512)], ones_row[:],
                                     bv_sb[:], start=False, stop=True)
            nc.vector.tensor_copy(
                v_all[:, 2 * t2 * VW:(2 * t2 + 2) * VW].rearrange(
                    "p (t h c) -> p t h c", h=8, c=2 * DK)[:, :, :, 0:DK],
                acc[:].rearrange("p (t h d) -> p t h d", h=8, d=DK))

        def emit_v_chunk(t, half=None, cell=None):
            if half in (None, 0):
                acc = ps_q.tile([P, 512], f32, tag="q", name="vacc")
                if cell is not None:
                    cell.append(acc)
            else:
                acc = cell.pop()
            k0 = 0 if half in (None, 0) else KT // 2
            k1 = KT if half in (None, 1) else KT // 2
            for kt in range(k0, k1):
                nc.tensor.matmul(
                    acc[:], xt[:, kt * T + t * P: kt * T + (t + 1) * P],
                    wv_sb[:, ts(kt, 512)],
                    start=(kt == 0),
                    stop=(not with_bias and kt == KT - 1))
            if half in (None, 1):
                if with_bias:
                    nc.tensor.matmul(acc[:], ones_row[:], bv_sb[:],
                                     start=False, stop=True)
                nc.vector.tensor_copy(
                    v_all[:, t * VW:(t + 1) * VW].rearrange(
                        "p (h c) -> p h c", h=8, c=2 * DK)[:, :, 0:DK],
                    acc[:].rearrange("p (h d) -> p h d", h=8, d=DK))

        op_parts = {}

        def emit_op(t, dc, half=None, cell=None, sc_evac=False):
            """out-projection for token tile t, output column half dc.
            half=0: kk 0..2 accumulated and parked in sbuf (bf16 partial);
            half=1: kk 3 matmul + add partial + evac.  Lets the bulk of the
            final tiles' work run before the last pair's normalize lands."""
            acc = ps_q.tile([P, 512], f32, tag="q", name="oacc")
            kks = range(4) if half is None else (
                range(3) if half == 0 else range(3, 4))
            for kk in kks:
                nc.tensor.matmul(
                    acc[:],
                    at[kk][t // 4][:, (t % 4) * P:(t % 4 + 1) * P],
                    wout_sb[:, kk * 1024 + dc * 512: kk * 1024 + dc * 512 + 512],
                    start=(kk == 0 or half == 1), stop=(kk == 3 or half == 0))
            if half == 0:
                part = oprt_pool.tile([P, 512], bf16, tag="op_part",
                                      name=f"part{t}_{dc}")
                nc.scalar.activation(part[:], acc[:],
                                     mybir.ActivationFunctionType.Identity)
                op_parts[(t, dc)] = part
                return
            o_sb = osb_pool.tile([P, 512], bf16, tag="o_sb")
            if half == 1:
                nc.vector.tensor_add(o_sb[:], acc[:], op_parts[(t, dc)][:])
            elif sc_evac:
                nc.scalar.activation(o_sb[:], acc[:],
                                     mybir.ActivationFunctionType.Identity)
            else:
                nc.vector.tensor_copy(o_sb[:], acc[:])
            # final tiles alternate DMA queues: halves the end-of-kernel
            # trigger serialization (scalar queue is free of exps by then)
            eng = nc.scalar if (sc_evac and (2 * t + dc) % 2) else nc.sync
            eng.dma_start(out_d[ts(t, P), ts(dc, 512)], o_sb[:])

        # ---- attention step machine ---------------------------------------
        # Globally software-pipelined: the scores for step i+1 (even across a
        # pair boundary) are issued while ScalarE computes exp(i), and fill
        # groups land inside the exp window, so neither engine waits.
        pair_ps = {}
        s_tiles = {}
        rec_tiles = {}

        def pair_begin(c, p):
            ta = ps_avr.tile([P, CH], f32, tag="avr", name="attnA")
            tb = ps_avr.tile([P, CH], f32, tag="avr", name="attnB")
            pair_ps[(c, p)] = (ta, tb)

        def emit_score(c, p, j):
            kq = qk[4 + p]
            qq = qk[p]
            off = max(0, P * (j - 4 * c))
            s_ps = ps_s.tile([P, 1024], f32, tag="s", name="s_ps")
            nc.tensor.matmul(
                s_ps[:, off:512], kq[0:DK, ts(j, P)],
                qq[0:DK, c * CH + off:(c + 1) * CH],
                start=True, stop=True)
            nc.tensor.matmul(
                s_ps[:, 512 + off:1024], kq[DK:P, ts(j, P)],
                qq[DK:P, c * CH + off:(c + 1) * CH],
                start=True, stop=True)
            s_tiles[(c, p, j)] = s_ps

        def emit_exp(c, p, j):
            off = max(0, P * (j - 4 * c))
            s_ps = s_tiles.pop((c, p, j))
            pt = pt_pool.tile([P, 1024], bf16, tag="pt")
            # [128, 2, n] views pairing the two head-halves (stride 512),
            # so trimmed exp / diagonal masking are single instructions
            pt3 = pt[:].rearrange("p (two n) -> p two n", two=2)
            s3 = s_ps[:].rearrange("p (two n) -> p two n", two=2)
            m3 = m01_blk[:, None, :].broadcast_to([P, 2, P])
            if j > 4 * c:  # diagonal block, trimmed
                nc.scalar.activation(
                    pt3[:, :, off:512], s3[:, :, off:512],
                    EXP, bias=neg12[:], scale=1.0)
                nc.vector.tensor_mul(
                    pt3[:, :, off:off + P], pt3[:, :, off:off + P], m3)
            elif j == 4 * c:  # diagonal block at chunk start
                nc.scalar.activation(
                    pt[:], s_ps[:], EXP, bias=neg12[:], scale=1.0)
                nc.vector.tensor_mul(
                    pt3[:, :, 0:P], pt3[:, :, 0:P], m3)
            else:
                nc.scalar.activation(
                    pt[:], s_ps[:], EXP, bias=neg12[:], scale=1.0)
            return pt

        def emit_pv(c, p, j, pt):
            ta, tb = pair_ps[(c, p)]
            nki = 4 * (c + 1)
            st = (j == 0)
            sp = (j == nki - 1)
            off = max(0, P * (j - 4 * c))
            vb = j * VW
            # [v_h | ones x64] stationary: rows 0:64 attn, rows 64:128 the
            # softmax denominator replicated across partitions
            nc.tensor.matmul(
                ta[:, off:CH],
                v_all[:, vb + 2 * p * P: vb + (2 * p + 1) * P],
                pt[:, off:512],
                start=st, stop=sp, skip_group_check=True)
            nc.tensor.matmul(
                tb[:, off:CH],
                v_all[:, vb + (2 * p + 1) * P: vb + (2 * p + 2) * P],
                pt[:, 512 + off:1024],
                start=st, stop=sp, skip_group_check=True)

        def emit_tail(c, p):
            """normalize: partition-parallel reciprocals of the replicated
            denominators, then multiply into at_all.  DVE-only chain, so it
            runs concurrently with the next pair's scores/fills.  The
            reciprocals must start at partition base 0 (custom-DVE ops
            silently ignore nonzero bases): rows 0:64 compute junk recips of
            attn values that nothing reads."""
            ta, tb = pair_ps.pop((c, p))
            rs_sb = rs_pool.tile([P, 2 * CH], f32, tag="rs_sb")
            dst = at[p][c]
            nc.vector.reciprocal_approx_fast(rs_sb[:, 0:CH], ta[:, :])
            nc.vector.tensor_mul(dst[0:DK, :], ta[0:DK, :],
                                 rs_sb[DK:P, 0:CH])
            nc.vector.reciprocal_approx_fast(rs_sb[:, CH:2 * CH], tb[:, :])
            nc.vector.tensor_mul(dst[DK:P, :], tb[0:DK, :],
                                 rs_sb[DK:P, CH:2 * CH])

        def qkf(i, n):
            """two ~1us granules sharing one psum accumulator"""
            cell = []
            return [lambda h=h: emit_qk_chunk(i, n, half=h, cell=cell)
                    for h in range(2)]

        def vf(t):
            cell = []
            return [lambda h=h: emit_v_chunk(t, half=h, cell=cell)
                    for h in range(2)]

        def opf(tt):
            return [lambda t=t, dc=dc: emit_op(t, dc)
                    for t in tt for dc in range(2)]

        def opf_early(tt):
            return [lambda t=t, dc=dc: emit_op(t, dc, half=0)
                    for t in tt for dc in range(2)]

        # ---- prelude: only what pair (0,0) needs — chunk-0 of q0/k0 and
        # v tiles 0..3 (v accumulated in the idle "s" score pool) -----------
        nc.vector.memset(v4[:, :, DK:2 * DK], 1.0)
        cell0, cell4 = [], []
        emit_qk_chunk(0, 0, half=0, cell=cell0)
        emit_qk_chunk(4, 0, half=0, cell=cell4)
        emit_qk_chunk(0, 0, half=1, cell=cell0)
        emit_qk_chunk(4, 0, half=1, cell=cell4)

        # ---- main interleave.  qk projection chunk n of tile i is first
        # needed by pair (n, i%4), so each pair carries its successor's two
        # qk chunk-groups; v/out-proj groups fill the remaining slack, with
        # out-proj (no early deadline) pushed into the late exp-bound
        # chunks. ----------------------------------------------------------
        fills = {
            (0, 0): vf(2) + vf(3) + qkf(1, 0) + qkf(5, 0),
            (0, 1): qkf(2, 0) + qkf(6, 0),
            (0, 2): qkf(3, 0) + qkf(7, 0),
            (0, 3): qkf(0, 1) + qkf(4, 1) + vf(4) + vf(5) + vf(6) + vf(7),
            (1, 0): qkf(1, 1) + qkf(5, 1) + vf(8),
            (1, 1): qkf(2, 1) + qkf(6, 1) + vf(9),
            (1, 2): qkf(3, 1) + qkf(7, 1) + vf(10),
            (1, 3): qkf(0, 2) + qkf(4, 2) + vf(11),
            (2, 0): qkf(1, 2) + qkf(5, 2) + opf([0]),
            (2, 1): qkf(2, 2) + qkf(6, 2) + opf([1]),
            (2, 2): qkf(3, 2) + qkf(7, 2) + opf([2, 3]),
            (2, 3): qkf(0, 3) + qkf(4, 3) + vf(12) + vf(13) + opf([4]),
            (3, 0): vf(14) + vf(15) + qkf(1, 3) + qkf(5, 3) + opf([5]),
            (3, 1): qkf(2, 3) + qkf(6, 3) + opf([6]),
            (3, 2): qkf(3, 3) + qkf(7, 3) + opf([7, 8]),
            (3, 3): opf([9, 10, 11]),
        }
        steps = [(c, p, j)
                 for c in range(NCH) for p in range(4)
                 for j in range(4 * (c + 1))]
        pair_begin(0, 0)
        emit_score(0, 0, 0)
        emit_v_merged(0)
        fcur = {}
        for idx, (c, p, j) in enumerate(steps):
            nki = 4 * (c + 1)
            pt = emit_exp(c, p, j)
            if idx + 1 < len(steps):
                nc2, np2, nj2 = steps[idx + 1]
                if nj2 == 0:
                    pair_begin(nc2, np2)
                emit_score(nc2, np2, nj2)
            # fills must complete by step nki-2: the last step's lookahead
            # score reads qk chunks that this pair's fills produce
            fl = fills.get((c, p), ())
            want = min(len(fl), max((j + 2) * len(fl) // nki,
                                    1 if j == 0 else 0))
            cur = fcur.get((c, p), 0)
            while cur < want:
                fl[cur]()
                cur += 1
            fcur[(c, p)] = cur
            emit_pv(c, p, j, pt)
            if j == nki - 1:
                emit_tail(c, p)
        for t in range(12, 16):
            for dc in range(2):
                emit_op(t, dc, sc_evac=True)

    nc.compile()
    return nc


def _get_program(with_bias):
    key = ("nc", with_bias)
    if key not in _CACHE:
        _CACHE[key] = _build_program(with_bias)
    return _CACHE[key]


def _prep_core_inputs(x, attn_mask, Wqkv, bqkv, Wout):
    """Per-core host-side sharding + DMA-friendly layouts."""
    # partial diagonal block: m01[ki_rel, qi_rel] = 1 iff qi_rel >= ki_rel
    m01 = np.triu(np.ones((P, P), np.float32)).astype(BF16)

    in_maps = []
    for core in range(NCORES):
        b, g = core // 2, core % 2
        xt = np.ascontiguousarray(
            x[b].T.reshape(KT, P, T).transpose(1, 0, 2).reshape(P, KT * T)
        ).astype(BF16)
        wq = Wqkv[:, 512 * g:512 * g + 512] * np.float32(0.125)
        wk = Wqkv[:, 1024 + 512 * g:1024 + 512 * g + 512]
        wqk = np.concatenate([wq, wk], axis=1)  # [1024, 1024]
        wqk = np.ascontiguousarray(
            wqk.reshape(KT, P, 8, P).transpose(1, 2, 0, 3).reshape(P, 8192)
        ).astype(BF16)
        wv = Wqkv[:, 2048 + 512 * g:2048 + 512 * g + 512]
        wv = np.ascontiguousarray(
            wv.reshape(KT, P, 512).transpose(1, 0, 2).reshape(P, KT * 512)
        ).astype(BF16)
        wo = Wout[512 * g:512 * g + 512, :]
        wo = np.ascontiguousarray(
            wo.reshape(4, P, 1024).transpose(1, 0, 2).reshape(P, 4096)
        ).astype(BF16)
        bq = bqkv[512 * g:512 * g + 512] * np.float32(0.125)
        bk = bqkv[1024 + 512 * g:1024 + 512 * g + 512]
        bqk = np.ascontiguousarray(
            np.concatenate([bq, bk]).reshape(8, P).T)
        bv = np.ascontiguousarray(
            bqkv[2048 + 512 * g:2048 + 512 * g + 512].reshape(1, 512)
        ).astype(BF16)
        in_maps.append({"xt": xt, "wqk": wqk, "wv": wv, "wout": wo,
                        "m01": m01, "bqk": bqk, "bv": bv})
    return in_maps


def _mask_is_causal(attn_mask):
    zero = (attn_mask == 0.0)
    if not np.array_equal(zero, np.tril(np.ones((T, T), dtype=bool))):
        return False
    return bool(np.all(attn_mask[~zero] <= np.float32(-50.0)))


def _numpy_fallback(x, attn_mask, Wqkv, bqkv, Wout, bout):
    qkv = x @ Wqkv + bqkv
    qkv = qkv.reshape(B, T, 3, H, DK).transpose(2, 0, 3, 1, 4)
    q, k, vv = qkv[0], qkv[1], qkv[2]
    scores = np.einsum("bhqd,bhkd->bhqk", q, k) / np.float32(np.sqrt(DK))
    scores = scores + attn_mask
    scores -= scores.max(axis=-1, keepdims=True)
    e = np.exp(scores)
    probs = e / e.sum(axis=-1, keepdims=True)
    attn = np.einsum("bhqk,bhkd->bhqd", probs, vv)
    attn = attn.transpose(0, 2, 1, 3).reshape(B, T, D)
    return (attn @ Wout + bout).astype(np.float32)


def _run(inputs, trace=False):
    from concourse.bass_utils import run_bass_kernel_spmd

    x = np.asarray(inputs["x"], dtype=np.float32)
    attn_mask = np.asarray(inputs["attn_mask"], dtype=np.float32)
    Wqkv = np.asarray(inputs["Wqkv"], dtype=np.float32)
    bqkv = np.asarray(inputs["bqkv"], dtype=np.float32)
    Wout = np.asarray(inputs["Wout"], dtype=np.float32)
    bout = np.asarray(inputs["bout"], dtype=np.float32)

    if not _mask_is_causal(attn_mask):
        return _numpy_fallback(x, attn_mask, Wqkv, bqkv, Wout, bout), None

    with_bias = bool(np.any(bqkv != 0.0))
    nc = _get_program(with_bias)
    in_maps = _prep_core_inputs(x, attn_mask, Wqkv, bqkv, Wout)
    res = run_bass_kernel_spmd(nc, in_maps, list(range(NCORES)), trace=trace)
    out = np.empty((B, T, D), np.float32)
    for b in range(B):
        out[b] = (res.results[2 * b]["out"].astype(np.float32)
                  + res.results[2 * b + 1]["out"].astype(np.float32) + bout)
    return out, res.exec_time_ns


def kernel(**inputs) -> np.ndarray:
    out, _ = _run(inputs, trace=False)
    return out



# revision 49
# speedup vs baseline: 1.0161x; 1.0143x over previous
"""Masked multi-head self-attention on 8 Trainium2 NeuronCores.

Sharding: core c handles batch b = c // 2 and head-group g = c % 2
(8 of 16 heads).  Data-parallel over B, tensor-parallel over heads for
qkv_proj (column split) / out_proj (row split).  The [T,T] causal mask
is exploited structurally (tile skipping); the host verifies the mask
is causal and falls back to numpy otherwise.  Host sums the two
head-group partial outputs per batch and adds bout.

Schedule: projection matmul groups are interleaved into the attention
stream so TensorE works through softmax (ScalarE) stretches.  The
softmax denominators ride free on the PV matmuls: each 64-wide V
stationary is widened to 128 with 64 all-ones columns (matmul cost
depends only on moving free size), so rows 64:128 of the PV psum hold
the denominator replicated across partitions.  The per-pair tail is
then a DVE-only chain - partition-parallel reciprocal + multiply into
the per-(pair,chunk) attention tiles - with no extra TensorE work and
no cross-partition broadcast.
"""

import numpy as np
import ml_dtypes

BF16 = ml_dtypes.bfloat16

B = 4
T = 2048
D = 1024
H = 16
DK = 64
P = 128
NCORES = 8

KT = D // P            # 8   k-tiles over d_model
TTILES = T // P        # 16  tiles over tokens
NCH = 4                # qi chunks of 512
CH = T // NCH          # 512

_CACHE = {}


def _build_program(with_bias=True):
    import concourse.bass as bass
    import concourse.tile as tile
    from concourse import bacc, mybir
    from contextlib import ExitStack

    f32 = mybir.dt.float32
    bf16 = mybir.dt.bfloat16
    nc = bacc.Bacc("TRN2", target_bir_lowering=False, debug=False,
                   num_devices=NCORES)

    xt_d = nc.declare_dram_parameter("xt", [P, KT * T], bf16, isOutput=False)
    wqk_d = nc.declare_dram_parameter("wqk", [P, 8 * 1024], bf16, isOutput=False)
    wv_d = nc.declare_dram_parameter("wv", [P, KT * 512], bf16, isOutput=False)
    wout_d = nc.declare_dram_parameter("wout", [P, 4 * 1024], bf16, isOutput=False)
    m01_d = nc.declare_dram_parameter("m01", [P, P], bf16, isOutput=False)
    bqk_d = nc.declare_dram_parameter("bqk", [P, 8], f32, isOutput=False)
    bv_d = nc.declare_dram_parameter("bv", [1, 512], bf16, isOutput=False)
    out_d = nc.declare_dram_parameter("out", [T, D], bf16, isOutput=True)

    ts = bass.ts
    EXP = mybir.ActivationFunctionType.Exp

    with tile.TileContext(nc) as tc, ExitStack() as top:
        const = top.enter_context(tc.tile_pool(name="const", bufs=1))
        big = top.enter_context(tc.tile_pool(name="big", bufs=1))
        wqk_pool = top.enter_context(tc.tile_pool(name="wqk", bufs=8))
        pt_pool = top.enter_context(tc.tile_pool(name="pt", bufs=10))
        rs_pool = top.enter_context(tc.tile_pool(name="rs", bufs=2))
        osb_pool = top.enter_context(tc.tile_pool(name="osb", bufs=4))
        oprt_pool = top.enter_context(tc.tile_pool(name="oprt", bufs=8))
        # PSUM: "s" 2x[128,1024]f32 = 4 banks, "avr" 2x[128,512] = 2, "q" 2
        ps_s = top.enter_context(tc.tile_pool(name="ps_s", bufs=2, space="PSUM"))
        ps_avr = top.enter_context(tc.tile_pool(name="ps_avr", bufs=2, space="PSUM"))
        ps_q = top.enter_context(tc.tile_pool(name="ps_q", bufs=2, space="PSUM"))

        ones_row = const.tile([1, P], bf16, tag="ones_row")
        neg12 = const.tile([P, 1], f32, tag="neg12")
        bqk_sb = const.tile([P, 8], f32, tag="bqk")
        bv_sb = const.tile([1, 512], bf16, tag="bv")
        m01_blk = const.tile([P, P], bf16, tag="m01")
        nc.vector.memset(ones_row[:], 1.0)
        nc.vector.memset(neg12[:], -12.0)

        def qk_evac(dst, acc, i):
            if with_bias:
                nc.vector.tensor_scalar_add(dst, acc, bqk_sb[:, i:i + 1])
            else:
                nc.vector.tensor_copy(dst, acc)

        # qk[i] for i<4: q of head pair i (pre-scaled 1/8); i>=4: k of pair i-4
        # v_all: per token tile, 8 head slots of 128 cols = [v_h | ones x64];
        # stationary width is free, so each PV matmul also produces the
        # softmax denominator replicated across partitions 64:128
        VW = 8 * 2 * DK  # 1024
        xt = big.tile([P, KT * T], bf16, tag="xt")
        qk = [big.tile([P, T], bf16, tag=f"qk{i}", name=f"qk{i}")
              for i in range(8)]
        v_all = big.tile([P, TTILES * VW], bf16, tag="v")
        # at[p][c]: one tile per (head-pair, token-chunk) so out-proj reads
        # depend only on the pair tail that actually wrote them
        at = [[big.tile([P, CH], bf16, tag=f"at{p}_{c}", name=f"at{p}_{c}")
               for c in range(NCH)] for p in range(4)]
        wv_sb = big.tile([P, KT * 512], bf16, tag="wv")
        wout_sb = big.tile([P, 4 * 1024], bf16, tag="wout")
        v4 = v_all[:].rearrange("p (t h c) -> p (t h) c", h=8, c=2 * DK)

        wqk_sb = {}

        def dma_wqk(i, eng=None, split=False):
            w = wqk_pool.tile([P, 1024], bf16, tag="wqk", name=f"wqk{i}")
            e = eng or nc.sync
            if split:
                e.dma_start(w[:, 0:512], wqk_d[:, i * 1024: i * 1024 + 512])
                e.dma_start(w[:, 512:1024],
                            wqk_d[:, i * 1024 + 512: (i + 1) * 1024])
            else:
                e.dma_start(w[:], wqk_d[:, ts(i, 1024)])
            wqk_sb[i] = w

        if with_bias:
            nc.sync.dma_start(bqk_sb[:], bqk_d[:])
        # DMA program follows matmul consumption order, interleaved on both
        # HWDGE queues: qk(0,0)h0 wants wqk0+quarters 0-3, qk(4,0) wants
        # wqk4, the v-prelude wants wv, exp(0,0,0) wants m01, then chunk-0
        # fills want wqk1/5, chunk-1+ wants the xt second halves + wqk2/6/3/7
        dma_wqk(0)
        for kt in range(4):
            eng = nc.scalar if kt % 2 == 0 else nc.sync
            eng.dma_start(xt[:, kt * T: kt * T + 512],
                          xt_d[:, kt * T: kt * T + 512])
        dma_wqk(4, nc.scalar)
        for kt in range(4, KT):
            eng = nc.scalar if kt % 2 == 0 else nc.sync
            eng.dma_start(xt[:, kt * T: kt * T + 512],
                          xt_d[:, kt * T: kt * T + 512])
        nc.sync.dma_start(wv_sb[:, 0:4 * 512], wv_d[:, 0:4 * 512])
        nc.scalar.dma_start(wv_sb[:, 4 * 512:], wv_d[:, 4 * 512:])
        nc.sync.dma_start(m01_blk[:], m01_d[:])
        if with_bias:
            nc.sync.dma_start(bv_sb[:], bv_d[:])
        dma_wqk(1)
        dma_wqk(5, nc.scalar)
        for kt in range(KT):
            eng = nc.scalar if kt % 2 == 0 else nc.sync
            eng.dma_start(xt[:, kt * T + 512: (kt + 1) * T],
                          xt_d[:, kt * T + 512: (kt + 1) * T])
        nc.sync.dma_start(wout_sb[:], wout_d[:])
        dma_wqk(2)
        dma_wqk(6, nc.scalar)
        dma_wqk(3)
        dma_wqk(7, nc.scalar)

        def emit_qk_chunk(i, n, half=None, cell=None):
            """half=None: whole 8-matmul chain; half=0/1: split granule
            (the two granules share one psum tile via `cell`)."""
            if half in (None, 0):
                acc = ps_q.tile([P, 512], f32, tag="q", name="qkacc")
                if cell is not None:
                    cell.append(acc)
            else:
                acc = cell.pop()
            k0 = 0 if half in (None, 0) else KT // 2
            k1 = KT if half in (None, 1) else KT // 2
            for kt in range(k0, k1):
                nc.tensor.matmul(
                    acc[:], wqk_sb[i][:, ts(kt, P)],
                    xt[:, kt * T + n * CH: kt * T + (n + 1) * CH],
                    start=(kt == 0), stop=(kt == KT - 1))
            if half in (None, 1):
                qk_evac(qk[i][:, ts(n, CH)], acc[:], i)

        def emit_v_merged(t2):
            """v token-tiles 2*t2, 2*t2+1 in one 2-bank psum tile"""
            acc = ps_s.tile([P, 1024], f32, tag="s", name="vacc")
            for kt in range(KT):
                for h in range(2):
                    t = 2 * t2 + h
                    nc.tensor.matmul(
                        acc[:, ts(h, 512)],
                        xt[:, kt * T + t * P: kt * T + (t + 1) * P],
                        wv_sb[:, ts(kt, 512)],
                        start=(kt == 0),
                        stop=(not with_bias and kt == KT - 1))
            if with_bias:
                for h in range(2):
                    nc.tensor.matmul(acc[:, ts(h, 512)], ones_row[:],
                                     bv_sb[:], start=False, stop=True)
            nc.vector.tensor_copy(
                v_all[:, 2 * t2 * VW:(2 * t2 + 2) * VW].rearrange(
                    "p (t h c) -> p t h c", h=8, c=2 * DK)[:, :, :, 0:DK],
                acc[:].rearrange("p (t h d) -> p t h d", h=8, d=DK))

        def emit_v_chunk(t, half=None, cell=None):
            if half in (None, 0):
                acc = ps_q.tile([P, 512], f32, tag="q", name="vacc")
                if cell is not None:
                    cell.append(acc)
            else:
                acc = cell.pop()
            k0 = 0 if half in (None, 0) else KT // 2
            k1 = KT if half in (None, 1) else KT // 2
            for kt in range(k0, k1):
                nc.tensor.matmul(
                    acc[:], xt[:, kt * T + t * P: kt * T + (t + 1) * P],
                    wv_sb[:, ts(kt, 512)],
                    start=(kt == 0),
                    stop=(not with_bias and kt == KT - 1))
            if half in (None, 1):
                if with_bias:
                    nc.tensor.matmul(acc[:], ones_row[:], bv_sb[:],
                                     start=False, stop=True)
                nc.vector.tensor_copy(
                    v_all[:, t * VW:(t + 1) * VW].rearrange(
                        "p (h c) -> p h c", h=8, c=2 * DK)[:, :, 0:DK],
                    acc[:].rearrange("p (h d) -> p h d", h=8, d=DK))

        op_parts = {}

        def emit_op(t, dc, half=None, cell=None, sc_evac=False):
            """out-projection for token tile t, output column half dc.
            half=0: kk 0..2 accumulated and parked in sbuf (bf16 partial);
            half=1: kk 3 matmul + add partial + evac.  Lets the bulk of the
            final tiles' work run before the last pair's normalize lands."""
            acc = ps_q.tile([P, 512], f32, tag="q", name="oacc")
            kks = range(4) if half is None else (
                range(3) if half == 0 else range(3, 4))
            for kk in kks:
                nc.tensor.matmul(
                    acc[:],
                    at[kk][t // 4][:, (t % 4) * P:(t % 4 + 1) * P],
                    wout_sb[:, kk * 1024 + dc * 512: kk * 1024 + dc * 512 + 512],
                    start=(kk == 0 or half == 1), stop=(kk == 3 or half == 0))
            if half == 0:
                part = oprt_pool.tile([P, 512], bf16, tag="op_part",
                                      name=f"part{t}_{dc}")
                nc.scalar.activation(part[:], acc[:],
                                     mybir.ActivationFunctionType.Identity)
                op_parts[(t, dc)] = part
                return
            o_sb = osb_pool.tile([P, 512], bf16, tag="o_sb")
            if half == 1:
                nc.vector.tensor_add(o_sb[:], acc[:], op_parts[(t, dc)][:])
            elif sc_evac:
                nc.scalar.activation(o_sb[:], acc[:],
                                     mybir.ActivationFunctionType.Identity)
            else:
                nc.vector.tensor_copy(o_sb[:], acc[:])
            # final tiles alternate DMA queues: halves the end-of-kernel
            # trigger serialization (scalar queue is free of exps by then)
            eng = nc.scalar if (sc_evac and (2 * t + dc) % 2) else nc.sync
            eng.dma_start(out_d[ts(t, P), ts(dc, 512)], o_sb[:])

        # ---- attention step machine ---------------------------------------
        # Globally software-pipelined: the scores for step i+1 (even across a
        # pair boundary) are issued while ScalarE computes exp(i), and fill
        # groups land inside the exp window, so neither engine waits.
        pair_ps = {}
        s_tiles = {}
        rec_tiles = {}

        def pair_begin(c, p):
            ta = ps_avr.tile([P, CH], f32, tag="avr", name="attnA")
            tb = ps_avr.tile([P, CH], f32, tag="avr", name="attnB")
            pair_ps[(c, p)] = (ta, tb)

        def emit_score(c, p, j):
            kq = qk[4 + p]
            qq = qk[p]
            off = max(0, P * (j - 4 * c))
            s_ps = ps_s.tile([P, 1024], f32, tag="s", name="s_ps")
            nc.tensor.matmul(
                s_ps[:, off:512], kq[0:DK, ts(j, P)],
                qq[0:DK, c * CH + off:(c + 1) * CH],
                start=True, stop=True)
            nc.tensor.matmul(
                s_ps[:, 512 + off:1024], kq[DK:P, ts(j, P)],
                qq[DK:P, c * CH + off:(c + 1) * CH],
                start=True, stop=True)
            s_tiles[(c, p, j)] = s_ps

        def emit_exp(c, p, j):
            off = max(0, P * (j - 4 * c))
            s_ps = s_tiles.pop((c, p, j))
            pt = pt_pool.tile([P, 1024], bf16, tag="pt")
            # [128, 2, n] views pairing the two head-halves (stride 512),
            # so trimmed exp / diagonal masking are single instructions
            pt3 = pt[:].rearrange("p (two n) -> p two n", two=2)
            s3 = s_ps[:].rearrange("p (two n) -> p two n", two=2)
            m3 = m01_blk[:, None, :].broadcast_to([P, 2, P])
            if j > 4 * c:  # diagonal block, trimmed
                nc.scalar.activation(
                    pt3[:, :, off:512], s3[:, :, off:512],
                    EXP, bias=neg12[:], scale=1.0)
                nc.vector.tensor_mul(
                    pt3[:, :, off:off + P], pt3[:, :, off:off + P], m3)
            elif j == 4 * c:  # diagonal block at chunk start
                nc.scalar.activation(
                    pt[:], s_ps[:], EXP, bias=neg12[:], scale=1.0)
                nc.vector.tensor_mul(
                    pt3[:, :, 0:P], pt3[:, :, 0:P], m3)
            else:
                nc.scalar.activation(
                    pt[:], s_ps[:], EXP, bias=neg12[:], scale=1.0)
            return pt

        def emit_pv(c, p, j, pt):
            ta, tb = pair_ps[(c, p)]
            nki = 4 * (c + 1)
            st = (j == 0)
            sp = (j == nki - 1)
            off = max(0, P * (j - 4 * c))
            vb = j * VW
            # [v_h | ones x64] stationary: rows 0:64 attn, rows 64:128 the
            # softmax denominator replicated across partitions
            nc.tensor.matmul(
                ta[:, off:CH],
                v_all[:, vb + 2 * p * P: vb + (2 * p + 1) * P],
                pt[:, off:512],
                start=st, stop=sp, skip_group_check=True)
            nc.tensor.matmul(
                tb[:, off:CH],
                v_all[:, vb + (2 * p + 1) * P: vb + (2 * p + 2) * P],
                pt[:, 512 + off:1024],
                start=st, stop=sp, skip_group_check=True)

        def emit_tail(c, p):
            """normalize: partition-parallel reciprocals of the replicated
            denominators, then multiply into at_all.  DVE-only chain, so it
            runs concurrently with the next pair's scores/fills.  The
            reciprocals must start at partition base 0 (custom-DVE ops
            silently ignore nonzero bases): rows 0:64 compute junk recips of
            attn values that nothing reads."""
            ta, tb = pair_ps.pop((c, p))
            rs_sb = rs_pool.tile([P, 2 * CH], f32, tag="rs_sb")
            dst = at[p][c]
            nc.vector.reciprocal_approx_fast(rs_sb[:, 0:CH], ta[:, :])
            nc.vector.tensor_mul(dst[0:DK, :], ta[0:DK, :],
                                 rs_sb[DK:P, 0:CH])
            nc.vector.reciprocal_approx_fast(rs_sb[:, CH:2 * CH], tb[:, :])
            nc.vector.tensor_mul(dst[DK:P, :], tb[0:DK, :],
                                 rs_sb[DK:P, CH:2 * CH])

        def qkf(i, n):
            """two ~1us granules sharing one psum accumulator"""
            cell = []
            return [lambda h=h: emit_qk_chunk(i, n, half=h, cell=cell)
                    for h in range(2)]

        def vf(t):
            cell = []
            return [lambda h=h: emit_v_chunk(t, half=h, cell=cell)
                    for h in range(2)]

        def opf(tt):
            return [lambda t=t, dc=dc: emit_op(t, dc)
                    for t in tt for dc in range(2)]

        def opf_early(tt):
            return [lambda t=t, dc=dc: emit_op(t, dc, half=0)
                    for t in tt for dc in range(2)]

        # ---- prelude: only what pair (0,0) needs — chunk-0 of q0/k0 and
        # v tiles 0..3 (v accumulated in the idle "s" score pool) -----------
        nc.vector.memset(v4[:, :, DK:2 * DK], 1.0)
        cell0, cell4 = [], []
        emit_qk_chunk(0, 0, half=0, cell=cell0)
        emit_qk_chunk(4, 0, half=0, cell=cell4)
        emit_qk_chunk(0, 0, half=1, cell=cell0)
        emit_qk_chunk(4, 0, half=1, cell=cell4)

        # ---- main interleave.  qk projection chunk n of tile i is first
        # needed by pair (n, i%4), so each pair carries its successor's two
        # qk chunk-groups; v/out-proj groups fill the remaining slack, with
        # out-proj (no early deadline) pushed into the late exp-bound
        # chunks. ----------------------------------------------------------
        fills = {
            (0, 0): vf(2) + vf(3) + qkf(1, 0) + qkf(5, 0),
            (0, 1): qkf(2, 0) + qkf(6, 0),
            (0, 2): qkf(3, 0) + qkf(7, 0),
            (0, 3): qkf(0, 1) + qkf(4, 1) + vf(4) + vf(5) + vf(6) + vf(7),
            (1, 0): qkf(1, 1) + qkf(5, 1) + vf(8),
            (1, 1): qkf(2, 1) + qkf(6, 1) + vf(9),
            (1, 2): qkf(3, 1) + qkf(7, 1) + vf(10),
            (1, 3): qkf(0, 2) + qkf(4, 2) + vf(11),
            (2, 0): qkf(1, 2) + qkf(5, 2) + opf([0]),
            (2, 1): qkf(2, 2) + qkf(6, 2) + opf([1]),
            (2, 2): qkf(3, 2) + qkf(7, 2) + opf([2, 3]),
            (2, 3): qkf(0, 3) + qkf(4, 3) + vf(12) + vf(13) + opf([4]),
            (3, 0): vf(14) + vf(15) + qkf(1, 3) + qkf(5, 3) + opf([5]),
            (3, 1): qkf(2, 3) + qkf(6, 3) + opf([6]),
            (3, 2): qkf(3, 3) + qkf(7, 3) + opf([7, 8]),
            (3, 3): opf([9, 10, 11]),
        }
        steps = [(c, p, j)
                 for c in range(NCH) for p in range(4)
                 for j in range(4 * (c + 1))]
        pair_begin(0, 0)
        emit_score(0, 0, 0)
        emit_v_merged(0)
        fcur = {}
        for idx, (c, p, j) in enumerate(steps):
            nki = 4 * (c + 1)
            pt = emit_exp(c, p, j)
            if idx + 1 < len(steps):
                nc2, np2, nj2 = steps[idx + 1]
                if nj2 == 0:
                    pair_begin(nc2, np2)
                emit_score(nc2, np2, nj2)
            # fills must complete by step nki-2: the last step's lookahead
            # score reads qk chunks that this pair's fills produce
            fl = fills.get((c, p), ())
            if (c, p) == (3, 3):
                # its op granules read at[3][2], written by tail(3,2) as
                # this pair starts: pace them a few steps in
                want = min(len(fl), max(0, (j - 1) * len(fl) // nki))
            else:
                want = min(len(fl), max((j + 2) * len(fl) // nki,
                                        1 if j == 0 else 0))
            cur = fcur.get((c, p), 0)
            while cur < want:
                fl[cur]()
                cur += 1
            fcur[(c, p)] = cur
            emit_pv(c, p, j, pt)
            if j == nki - 1:
                emit_tail(c, p)
        fl = fills[(3, 3)]
        for k in range(fcur[(3, 3)], len(fl)):
            fl[k]()
        for t in range(12, 16):
            for dc in range(2):
                emit_op(t, dc, sc_evac=True)

    nc.compile()
    return nc


def _get_program(with_bias):
    key = ("nc", with_bias)
    if key not in _CACHE:
        _CACHE[key] = _build_program(with_bias)
    return _CACHE[key]


def _prep_core_inputs(x, attn_mask, Wqkv, bqkv, Wout):
    """Per-core host-side sharding + DMA-friendly layouts."""
    # partial diagonal block: m01[ki_rel, qi_rel] = 1 iff qi_rel >= ki_rel
    m01 = np.triu(np.ones((P, P), np.float32)).astype(BF16)

    in_maps = []
    for core in range(NCORES):
        b, g = core // 2, core % 2
        xt = np.ascontiguousarray(
            x[b].T.reshape(KT, P, T).transpose(1, 0, 2).reshape(P, KT * T)
        ).astype(BF16)
        wq = Wqkv[:, 512 * g:512 * g + 512] * np.float32(0.125)
        wk = Wqkv[:, 1024 + 512 * g:1024 + 512 * g + 512]
        wqk = np.concatenate([wq, wk], axis=1)  # [1024, 1024]
        wqk = np.ascontiguousarray(
            wqk.reshape(KT, P, 8, P).transpose(1, 2, 0, 3).reshape(P, 8192)
        ).astype(BF16)
        wv = Wqkv[:, 2048 + 512 * g:2048 + 512 * g + 512]
        wv = np.ascontiguousarray(
            wv.reshape(KT, P, 512).transpose(1, 0, 2).reshape(P, KT * 512)
        ).astype(BF16)
        wo = Wout[512 * g:512 * g + 512, :]
        wo = np.ascontiguousarray(
            wo.reshape(4, P, 1024).transpose(1, 0, 2).reshape(P, 4096)
        ).astype(BF16)
        bq = bqkv[512 * g:512 * g + 512] * np.float32(0.125)
        bk = bqkv[1024 + 512 * g:1024 + 512 * g + 512]
        bqk = np.ascontiguousarray(
            np.concatenate([bq, bk]).reshape(8, P).T)
        bv = np.ascontiguousarray(
            bqkv[2048 + 512 * g:2048 + 512 * g + 512].reshape(1, 512)
        ).astype(BF16)
        in_maps.append({"xt": xt, "wqk": wqk, "wv": wv, "wout": wo,
                        "m01": m01, "bqk": bqk, "bv": bv})
    return in_maps


def _mask_is_causal(attn_mask):
    zero = (attn_mask == 0.0)
    if not np.array_equal(zero, np.tril(np.ones((T, T), dtype=bool))):
        return False
    return bool(np.all(attn_mask[~zero] <= np.float32(-50.0)))


def _numpy_fallback(x, attn_mask, Wqkv, bqkv, Wout, bout):
    qkv = x @ Wqkv + bqkv
    qkv = qkv.reshape(B, T, 3, H, DK).transpose(2, 0, 3, 1, 4)
    q, k, vv = qkv[0], qkv[1], qkv[2]
    scores = np.einsum("bhqd,bhkd->bhqk", q, k) / np.float32(np.sqrt(DK))
    scores = scores + attn_mask
    scores -= scores.max(axis=-1, keepdims=True)
    e = np.exp(scores)
    probs = e / e.sum(axis=-1, keepdims=True)
    attn = np.einsum("bhqk,bhkd->bhqd", probs, vv)
    attn = attn.transpose(0, 2, 1, 3).reshape(B, T, D)
    return (attn @ Wout + bout).astype(np.float32)


def _run(inputs, trace=False):
    from concourse.bass_utils import run_bass_kernel_spmd

    x = np.asarray(inputs["x"], dtype=np.float32)
    attn_mask = np.asarray(inputs["attn_mask"], dtype=np.float32)
    Wqkv = np.asarray(inputs["Wqkv"], dtype=np.float32)
    bqkv = np.asarray(inputs["bqkv"], dtype=np.float32)
    Wout = np.asarray(inputs["Wout"], dtype=np.float32)
    bout = np.asarray(inputs["bout"], dtype=np.float32)

    if not _mask_is_causal(attn_mask):
        return _numpy_fallback(x, attn_mask, Wqkv, bqkv, Wout, bout), None

    with_bias = bool(np.any(bqkv != 0.0))
    nc = _get_program(with_bias)
    in_maps = _prep_core_inputs(x, attn_mask, Wqkv, bqkv, Wout)
    res = run_bass_kernel_spmd(nc, in_maps, list(range(NCORES)), trace=trace)
    out = np.empty((B, T, D), np.float32)
    for b in range(B):
        out[b] = (res.results[2 * b]["out"].astype(np.float32)
                  + res.results[2 * b + 1]["out"].astype(np.float32) + bout)
    return out, res.exec_time_ns


def kernel(**inputs) -> np.ndarray:
    out, _ = _run(inputs, trace=False)
    return out

